# revision 2
# baseline (speedup 1.0000x reference)
"""AutoCorrelation kernel v3 for Trainium2 (Bass/Tile), 8-core data parallel.

Math shortcut (same as v1/v2): mean-over-lags of the circular correlation
factorizes, so x_corr_mean[b,l] = 1/(H*L) * sum_h (sum_d q)[l,h] * (sum_d k)[l,h].
Then top-6 over l, softmax, weighted sum of value rows -> [B,H,D].

Final version: chunk-major combined-batch design.  16 x 0.25MB pieces on
the single sync HWDGE ring (two rings measurably LOWER aggregate DMA
throughput; flat 128x2KB descriptors hit full stream rate), chunk-major so
both batches of a chunk arrive back to back.  Per chunk: one DVE reduce
covers both batches ([128,2,8,64]); k chunks 0-2 are pre-folded 64->32->16
on the otherwise-idle GpSimd then short-reduced on DVE; the last chunk
reduces per-piece directly on DVE so the post-stream chain has no GpSimd
hop.  One TT-mul + grouped reduce makes BOTH corr columns; one PE transpose
per chunk fills a [2,512] PSUM corr tile.  The top-k runs ONCE for both
batches: 2-channel MAX8/FIND (same cost as 1-channel), one stream
transpose, two 6-row fp16 indirect gathers (batch base via element_offset).
Weights: one ACT exp [2,6] + accum, normalized in f32 while the gathers
fly, PE-transposed [2,6]->[6,2] and cast fp16; two matmuls lhsT=wT16[:,b],
rhs=gath16[b]; ACT/DVE copies; two stores.
Empirical op costs (ntff): DVE reduce ~160+1.03*elems/lane ns, STT[128,8]
+accum 238, MAX8/FIND[2,512] 687/676, ST32x32 191, GpSimd fold 734/466
(1-chunk), ACT copy[2,512] ~710, PE transpose 420-520, gather issue ~1100.
"""

import numpy as np

import concourse.bass as bass
import concourse.mybir as mybir
import concourse.tile as tile
from concourse.masks import make_identity
from concourse.bass_utils import run_bass_kernel_spmd

B, L, H, D = 16, 512, 8, 64
HD = H * D                  # 512
NCORES = 8
BPC = B // NCORES           # 2 batches per core
ROWS = BPC * L              # 1024 rows of [HD] per core
P = 128
TPB = L // P                # 4 chunks per batch
KTOP = 6                    # k = int(log(512)) = 6
SCALE = 1.0 / (H * L)

_CACHE = {}


def _emit(tc, q, k, v, out):
    nc = tc.nc
    from contextlib import ExitStack

    with ExitStack() as ctx:
        main = ctx.enter_context(tc.tile_pool(name="main", bufs=1))
        small = ctx.enter_context(tc.tile_pool(name="small", bufs=1))
        psum = ctx.enter_context(tc.tile_pool(name="psum", bufs=1, space="PSUM"))

        # rows = b*L + t*P + p  ->  piece per chunk t carries both batches
        q4 = q.rearrange("(b t p) m -> t b p m", b=BPC, p=P)
        k4 = k.rearrange("(b t p) m -> t b p m", b=BPC, p=P)

        qt = main.tile([P, TPB, BPC, HD], mybir.dt.float32, tag="qt", name="qt")
        kt = main.tile([P, TPB, BPC, HD], mybir.dt.float32, tag="kt", name="kt")

        # ---- loads: 16 x 0.25MB pieces (one per tensor/chunk/batch; flat
        # 128x2KB descriptors hit full stream rate), single sync ring,
        # chunk-major so both batches of a chunk arrive back to back
        for t in range(TPB):
            for b in range(BPC):
                nc.sync.dma_start(
                    out=qt[:, t : t + 1, b : b + 1],
                    in_=q4[t : t + 1, b : b + 1].rearrange("t b p m -> p t b m"),
                )
            for b in range(BPC):
                nc.sync.dma_start(
                    out=kt[:, t : t + 1, b : b + 1],
                    in_=k4[t : t + 1, b : b + 1].rearrange("t b p m -> p t b m"),
                )

        ident = small.tile([P, P], mybir.dt.float32)
        make_identity(nc, ident[:])

        psum_corr = psum.tile([BPC, L], mybir.dt.float32, tag="pcorr", name="pcorr")
        psum_out = [
            psum.tile([1, HD], mybir.dt.float32, tag=f"pout{b}", name=f"pout{b}")
            for b in range(BPC)
        ]

        kf1 = small.tile([P, BPC * H, D // 2], mybir.dt.float32, tag="kf1", name="kf1")
        kf2 = small.tile([P, TPB, BPC * H, D // 4], mybir.dt.float32, tag="kf2", name="kf2")
        sq = small.tile([P, TPB, BPC, H], mybir.dt.float32, tag="sq", name="sq")
        sk = small.tile([P, TPB, BPC, H], mybir.dt.float32, tag="sk", name="sk")
        junk = [small.tile([P, BPC, H], mybir.dt.float32, tag=f"junk{i}", name=f"junk{i}") for i in range(2)]
        corr = small.tile([P, TPB * BPC], mybir.dt.float32, tag="corr", name="corr")
        maxv = small.tile([BPC, 8], mybir.dt.float32, tag="maxv", name="maxv")
        istage = small.tile([32, 32], mybir.dt.uint32, tag="ist", name="ist")
        istageT = small.tile([32, 32], mybir.dt.uint32, tag="istT", name="istT")
        wstage = small.tile([BPC, KTOP], mybir.dt.float32, tag="wst", name="wst")
        ssum = small.tile([BPC, 1], mybir.dt.float32, tag="ssum", name="ssum")
        rsum = small.tile([BPC, 1], mybir.dt.float32, tag="rsum", name="rsum")
        psum_wT = psum.tile([KTOP, BPC], mybir.dt.float32, tag="pwT", name="pwT")
        wT16 = small.tile([KTOP, BPC], mybir.dt.float16, tag="wT16", name="wT16")
        gath16 = [small.tile([KTOP, HD], mybir.dt.float16, tag=f"g16{b}", name=f"g16{b}") for b in range(BPC)]
        outt = [small.tile([1, HD], mybir.dt.float32, tag=f"ot{b}", name=f"ot{b}") for b in range(BPC)]
        # dummy exp pulls ACT_TABLE_LOAD into the stream window
        nc.scalar.activation(
            out=ssum[0:1, 0:1],
            in_=ssum[0:1, 0:1],
            func=mybir.ActivationFunctionType.Exp,
            scale=1.0,
        )

        def reduce_q(t):
            nc.vector.reduce_sum(
                out=sq[:, t : t + 1],
                in_=qt[:, t : t + 1].rearrange("p t b (h d) -> p (t b h) d", d=D),
                axis=mybir.AxisListType.X,
            )

        def reduce_q_piece(t, b):
            nc.vector.reduce_sum(
                out=sq[:, t : t + 1, b : b + 1],
                in_=qt[:, t : t + 1, b : b + 1].rearrange(
                    "p t b (h d) -> p (t b h) d", d=D
                ),
                axis=mybir.AxisListType.X,
            )

        def reduce_k_piece(t, b):
            nc.vector.reduce_sum(
                out=sk[:, t : t + 1, b : b + 1],
                in_=kt[:, t : t + 1, b : b + 1].rearrange(
                    "p t b (h d) -> p (t b h) d", d=D
                ),
                axis=mybir.AxisListType.X,
            )

        def fold_k(t):
            # GpSimd-only part: 64 -> 32 -> 16 along d for both batches
            xv = kt[:, t].rearrange("p b (h two d) -> p two (b h) d", two=2, d=D // 2)
            nc.gpsimd.tensor_add(kf1[:], xv[:, 0], xv[:, 1])
            f2 = kf1.rearrange("p g (two d) -> p two g d", two=2)
            nc.gpsimd.tensor_add(kf2[:, t], f2[:, 0], f2[:, 1])

        def reduce_k_folded(t):
            nc.vector.reduce_sum(
                out=sk[:, t : t + 1],
                in_=kf2[:, t : t + 1],
                axis=mybir.AxisListType.X,
            )

        def corr_cols(t):
            # one mul + one grouped reduce produce BOTH batches' corr columns
            nc.vector.tensor_mul(junk[t % 2][:], sq[:, t], sk[:, t])
            nc.vector.reduce_sum(
                out=corr[:, BPC * t : BPC * (t + 1)],
                in_=junk[t % 2][:],
                axis=mybir.AxisListType.X,
            )
            nc.tensor.transpose(
                out=psum_corr[:, P * t : P * (t + 1)],
                in_=corr[:, BPC * t : BPC * (t + 1)],
                identity=ident[:],
            )

        # ---- pipeline (scheduler orders by its sim; total engine work is
        # the binding constraint, so emission order is best-effort only)
        fold_k(0)
        reduce_q(0)
        fold_k(1)
        reduce_q(1)
        reduce_k_folded(0)
        corr_cols(0)
        fold_k(2)
        reduce_q(2)
        reduce_k_folded(1)
        corr_cols(1)
        reduce_q_piece(3, 0)
        reduce_q_piece(3, 1)
        reduce_k_folded(2)
        corr_cols(2)
        reduce_k_piece(3, 0)
        reduce_k_piece(3, 1)
        corr_cols(3)

        # ---- one combined tail
        nc.vector.max(out=maxv[:], in_=psum_corr[:])
        # weights path starts as soon as maxv exists; it hides under the
        # FIND/transpose/gather chain below
        nc.scalar.activation(
            out=wstage[:],
            in_=maxv[:, 0:KTOP],
            func=mybir.ActivationFunctionType.Exp,
            scale=SCALE,
            accum_out=ssum[:],
        )
        nc.vector.max_index(
            out=istage[0:BPC, 0:8], in_max=maxv[:], in_values=psum_corr[:]
        )
        nc.vector.transpose(out=istageT[:], in_=istage[:])
        # two 6-row gathers (proven form); batch base via element_offset
        for b in range(BPC):
            nc.gpsimd.indirect_dma_start(
                out=gath16[b][:],
                out_offset=None,
                in_=v,
                in_offset=bass.IndirectOffsetOnAxis(
                    ap=istageT[0:KTOP, b : b + 1], axis=0
                ),
                element_offset=b * L * HD,
            )
        # normalize in f32 while the gather flies, then transpose the [2,6]
        # weight rows into [6,2] columns on the PE and cast fp16
        nc.vector.reciprocal(out=rsum[:], in_=ssum[:])
        nc.scalar.mul(wstage[:], wstage[:], rsum[:, 0:1])
        nc.tensor.transpose(
            out=psum_wT[:], in_=wstage[:], identity=ident[0:BPC, 0:BPC]
        )
        nc.scalar.copy(wT16[:], psum_wT[:])
        for b in range(BPC):
            nc.tensor.matmul(
                out=psum_out[b][:],
                lhsT=wT16[:, b : b + 1],
                rhs=gath16[b][:],
                start=True,
                stop=True,
            )
        nc.scalar.copy(outt[0][:], psum_out[0][:])
        nc.sync.dma_start(out=out[0:1, :], in_=outt[0][:])
        nc.vector.tensor_copy(outt[1][:], psum_out[1][:])
        nc.sync.dma_start(out=out[1:2, :], in_=outt[1][:])


def _build_bass():
    import concourse.bacc as bacc

    nc = bacc.Bacc(trn_type="TRN2", target_bir_lowering=False, debug=False)
    q = nc.dram_tensor("q", [ROWS, HD], mybir.dt.float32, kind="ExternalInput").ap()
    k = nc.dram_tensor("k", [ROWS, HD], mybir.dt.float32, kind="ExternalInput").ap()
    v = nc.dram_tensor("v", [ROWS, HD], mybir.dt.float32, kind="ExternalInput").ap()
    out = nc.dram_tensor(
        "out", [BPC, HD], mybir.dt.float32, kind="ExternalOutput"
    ).ap()
    with tile.TileContext(nc) as tc:
        _emit(tc, q, k, v, out)
    nc.compile()
    return nc


def _get_nc():
    if "nc" not in _CACHE:
        _CACHE["nc"] = _build_bass()
    return _CACHE["nc"]


def run_sharded(queries, keys, values, trace=False, **kw):
    """Shard over 8 cores, run, gather. Returns (out [16,8,64], BassKernelResults)."""
    nc = _get_nc()
    q = np.ascontiguousarray(np.asarray(queries, dtype=np.float32))
    k = np.ascontiguousarray(np.asarray(keys, dtype=np.float32))
    v = np.ascontiguousarray(np.asarray(values, dtype=np.float32))
    in_maps = []
    for c in range(NCORES):
        sl = slice(c * BPC, (c + 1) * BPC)
        in_maps.append(
            {
                "q": q[sl].reshape(ROWS, HD),
                "k": k[sl].reshape(ROWS, HD),
                "v": v[sl].reshape(ROWS, HD),
            }
        )
    res = run_bass_kernel_spmd(nc, in_maps, list(range(NCORES)), trace=trace, **kw)
    out = np.empty((B, H, D), dtype=np.float32)
    for c in range(NCORES):
        out[c * BPC : (c + 1) * BPC] = res.results[c]["out"].reshape(BPC, H, D)
    return out, res


def kernel(queries, keys, values, B=None, **_ignored):
    out, _ = run_sharded(queries, keys, values, trace=False)
    return out
